# revision 6
# baseline (speedup 1.0000x reference)
"""Trainium2 Bass kernel for Llama-style GQA attention (B=2,S=2048,H=4096,NH=32,NKV=8,HD=128).

Sharding: tensor-parallel over heads — core c owns Q-heads 4c..4c+3 and GQA KV-head c
(Wq/Wk/Wv column-parallel, Wo row-parallel), ReduceScatter over token rows for the
output projection. kernel(**inputs) takes full inputs, returns the full output.
"""

import math
import os
from contextlib import ExitStack

import numpy as np

B, S, H = 2, 2048, 4096
NH, NKV, HD = 32, 8, 128
THETA = 1000000.0
NCORES = 8
QH = NH // NCORES            # 4 q-heads per core
TOK = B * S                  # 4096 tokens (flattened batch*seq)
QO = QH * HD                 # 512 q-out dims per core
TT = TOK // 128              # 32 token tiles of 128
TS = TOK // 512              # 8 token slices of 512
SB = S // 512                # 4 q-slices of 512 per batch
KTB = S // 128               # 16 k-tiles of 128 per batch
F32 = None                   # set after mybir import

LAST_EXEC_NS = None
LAST_RESULT = None

_compiled = {}


def _build():
    import concourse.bass as bass
    import concourse.mybir as mybir
    import concourse.tile as tile
    from concourse import bacc

    f32 = mybir.dt.float32
    nc = bacc.Bacc("TRN2", target_bir_lowering=False, debug=False,
                   num_devices=NCORES)

    def inp(name, shape):
        return nc.dram_tensor(name, shape, f32, kind="ExternalInput").ap()

    xT = inp("xT", (H, TOK))            # hidden transposed
    wqT = inp("wqT", (H, QO))           # Wq shard, pre-transposed (k=h on rows)
    wkT = inp("wkT", (H, HD))
    wvT = inp("wvT", (H, HD))
    woT = inp("woT", (QO, H))           # Wo shard transposed (k=o on rows)
    bqP = inp("bqP", (128, QH))         # bq shard as [d, head]
    bkP = inp("bkP", (128, 1))
    bvP = inp("bvP", (128, 1))
    bo8 = inp("bo8", (1, H))            # bo / 8
    cosT = inp("cosT", (HD, TOK))
    sinT = inp("sinT", (HD, TOK))
    rotM = inp("rotM", (HD, HD))        # lhsT for rotate_half_interleaved
    ident = inp("ident", (128, 128))
    ones = inp("ones", (128, 128))

    out = nc.dram_tensor("out", (TOK // NCORES, H), f32, kind="ExternalOutput").ap()
    partial = nc.dram_tensor("partial", (TOK, H), f32, kind="Internal").ap()
    rs_out = nc.dram_tensor("rs_out", (TOK // NCORES, H), f32,
                            kind="Internal").ap()

    inv_sqrt_hd = 1.0 / math.sqrt(HD)

    with tile.TileContext(nc) as tc, ExitStack() as stk:
        # ---------------- constants + persistent activations ----------------
        cpool = stk.enter_context(tc.tile_pool(name="consts", bufs=1))
        apool = stk.enter_context(tc.tile_pool(name="acts", bufs=1))

        cos_sb = cpool.tile([128, TOK], f32)
        nc.sync.dma_start(cos_sb[:], cosT[:])
        sin_sb = cpool.tile([128, TOK], f32)
        nc.sync.dma_start(sin_sb[:], sinT[:])
        rot_sb = cpool.tile([128, 128], f32)
        nc.sync.dma_start(rot_sb[:], rotM[:])
        id_sb = cpool.tile([128, 128], f32)
        nc.sync.dma_start(id_sb[:], ident[:])
        ones_sb = cpool.tile([128, 128], f32)
        nc.sync.dma_start(ones_sb[:], ones[:])
        bq_sb = cpool.tile([128, QH], f32)
        nc.sync.dma_start(bq_sb[:], bqP[:])
        bk_sb = cpool.tile([128, 1], f32)
        nc.sync.dma_start(bk_sb[:], bkP[:])
        bv_sb = cpool.tile([128, 1], f32)
        nc.sync.dma_start(bv_sb[:], bvP[:])
        bo8_sb = cpool.tile([1, H], f32)
        nc.sync.dma_start(bo8_sb[:], bo8[:])
        # bo/8 broadcast to all partitions, used in the O-proj PSUM drain
        bo_bc = cpool.tile([128, H], f32)
        nc.gpsimd.partition_broadcast(bo_bc[:], bo8_sb[:])

        # causal masks for the 4 diagonal 128x512 tiles: keep iff n - p - 128a >= 0
        mask_sb = cpool.tile([128, 4, 512], f32)
        nc.gpsimd.memset(mask_sb[:], 1.0)
        nc.gpsimd.affine_select(
            out=mask_sb[:], in_=mask_sb[:],
            compare_op=mybir.AluOpType.is_ge, fill=0.0, base=0,
            pattern=[[-128, 4], [1, 512]], channel_multiplier=-1,
        )

        KT = apool.tile([128, TOK], f32)        # K^T (rope'd), grows causally
        Vsb = apool.tile([128, TT, 128], f32)   # V in [t mod 128, t tile, d]

        # single fused causal loop over 512-token slices; one shared PSUM tag
        sp = stk.enter_context(tc.tile_pool(name="streams", bufs=3))
        tp = stk.enter_context(tc.tile_pool(name="tmps", bufs=2))
        qtp = stk.enter_context(tc.tile_pool(name="qts", bufs=2))
        vtp = stk.enter_context(tc.tile_pool(name="vts", bufs=2))
        atp = stk.enter_context(tc.tile_pool(name="attw", bufs=4))
        smp = stk.enter_context(tc.tile_pool(name="smalls", bufs=2))
        anp = stk.enter_context(tc.tile_pool(name="atn", bufs=2))
        stp = stk.enter_context(tc.tile_pool(name="ostage", bufs=2))
        wop = stk.enter_context(tc.tile_pool(name="wo", bufs=2))
        pp = stk.enter_context(tc.tile_pool(name="ps", bufs=8, space="PSUM"))

        def ps_tile(shape=(128, 512)):
            return pp.tile(list(shape), f32, name="ps", tag="ps")

        for ti in range(TS):
            b, j = ti // SB, ti % SB
            t0 = ti * 512
            # ---- QKV projection for this token slice (accumulate over h) ----
            psq = [ps_tile() for _ in range(QH)]
            psk = ps_tile()
            psv = ps_tile()
            for hi in range(H // 128):
                h0 = hi * 128
                xt = sp.tile([128, 512], f32, name="xt")
                nc.sync.dma_start(xt[:], xT[h0:h0 + 128, t0:t0 + 512])
                wq_t = sp.tile([128, QO], f32, name="wq_t")
                nc.sync.dma_start(wq_t[:], wqT[h0:h0 + 128, :])
                wk_t = sp.tile([128, HD], f32, name="wk_t")
                nc.sync.dma_start(wk_t[:], wkT[h0:h0 + 128, :])
                wv_t = sp.tile([128, HD], f32, name="wv_t")
                nc.sync.dma_start(wv_t[:], wvT[h0:h0 + 128, :])
                st = (hi == 0)
                en = (hi == H // 128 - 1)
                for q in range(QH):
                    nc.tensor.matmul(psq[q][:], wq_t[:, q * 128:(q + 1) * 128],
                                     xt[:], start=st, stop=en)
                nc.tensor.matmul(psk[:], wk_t[:], xt[:], start=st, stop=en)
                nc.tensor.matmul(psv[:], wv_t[:], xt[:], start=st, stop=en)

            # bias add (per-partition) while draining PSUM
            QTs = qtp.tile([128, QH, 512], f32, name="QTs")
            VTs = vtp.tile([128, 512], f32, name="VTs")
            for q in range(QH):
                nc.scalar.add(QTs[:, q, :], psq[q][:], bq_sb[:, q:q + 1])
            nc.scalar.add(KT[:, t0:t0 + 512], psk[:], bk_sb[:, 0:1])
            nc.scalar.add(VTs[:], psv[:], bv_sb[:, 0:1])

            # rope in place on QT / KT slices
            def rope(ap_slice):
                rps = ps_tile()
                nc.tensor.matmul(rps[:], rot_sb[:], ap_slice, start=True, stop=True)
                t1 = tp.tile([128, 512], f32, name="t1")
                nc.vector.tensor_mul(t1[:], ap_slice, cos_sb[:, t0:t0 + 512])
                t2 = tp.tile([128, 512], f32, name="t2")
                nc.vector.tensor_mul(t2[:], rps[:], sin_sb[:, t0:t0 + 512])
                nc.vector.tensor_add(ap_slice, t1[:], t2[:])

            for q in range(QH):
                rope(QTs[:, q, :])
            rope(KT[:, t0:t0 + 512])

            # V^T -> V (PE transpose of 128x128 blocks)
            for s4 in range(4):
                g = ti * 4 + s4
                vps = ps_tile((128, 128))
                nc.tensor.transpose(vps[:], VTs[:, s4 * 128:(s4 + 1) * 128],
                                    id_sb[:])
                nc.scalar.copy(Vsb[:, g, :], vps[:])

            # ---- causal attention for this q slice ----
            nk = 4 * j + 4                # k tiles of 128 within batch b
            ATn = anp.tile([128, QH, 512], f32, name="ATn")
            for h in range(QH):
                dn_ps = ps_tile((1, 512))
                at_ps = ps_tile()
                for ki in range(nk):
                    kg = b * KTB + ki
                    sc_ps = ps_tile()
                    nc.tensor.matmul(sc_ps[:], KT[:, kg * 128:(kg + 1) * 128],
                                     QTs[:, h, :], start=True, stop=True)
                    a_sb = atp.tile([128, 512], f32, name="a_sb")
                    nc.scalar.activation(a_sb[:], sc_ps[:],
                                         mybir.ActivationFunctionType.Exp,
                                         scale=inv_sqrt_hd)
                    if ki >= 4 * j:
                        nc.vector.tensor_mul(a_sb[:], a_sb[:],
                                             mask_sb[:, ki - 4 * j, :])
                    nc.tensor.matmul(dn_ps[:], ones_sb[:, 0:1], a_sb[:],
                                     start=(ki == 0), stop=(ki == nk - 1))
                    nc.tensor.matmul(at_ps[:], Vsb[:, kg, :], a_sb[:],
                                     start=(ki == 0), stop=(ki == nk - 1))
                dr = smp.tile([1, 512], f32, name="dr")
                nc.vector.reciprocal(dr[:], dn_ps[:])
                rb = smp.tile([128, 512], f32, name="rb")
                nc.gpsimd.partition_broadcast(rb[:], dr[:])
                nc.vector.tensor_mul(ATn[:, h, :], at_ps[:], rb[:])

            # ---- O-projection for this 512-token slice (row-parallel Wo) ----
            for f in range(H // 512):
                f0 = f * 512
                wo_t = wop.tile([128, QH, 512], f32, name="wo_t")
                for o in range(QH):
                    nc.sync.dma_start(wo_t[:, o, :],
                                      woT[o * 128:(o + 1) * 128, f0:f0 + 512])
                for t4 in range(4):
                    op_ps = ps_tile()
                    for o in range(QH):
                        nc.tensor.matmul(
                            op_ps[:], ATn[:, o, t4 * 128:(t4 + 1) * 128],
                            wo_t[:, o, :], start=(o == 0), stop=(o == QH - 1))
                    st_t = stp.tile([128, 512], f32, name="st_t")
                    nc.vector.tensor_add(st_t[:], op_ps[:], bo_bc[:, f0:f0 + 512])
                    nc.sync.dma_start(
                        partial[t0 + t4 * 128:t0 + (t4 + 1) * 128, f0:f0 + 512],
                        st_t[:])

        # ---------------- phase 3: reduce-scatter + output -------------------
        nc.gpsimd.collective_compute(
            "ReduceScatter", mybir.AluOpType.add,
            replica_groups=[list(range(NCORES))],
            ins=[partial.opt()], outs=[rs_out.opt()],
        )
        nc.sync.dma_start(out[:], rs_out[:])

    nc.compile()
    return nc


def _host_inputs(hidden_states, position_ids, Wq, bq, Wk, bk, Wv, bv, Wo, bo):
    f = np.float32
    X = np.asarray(hidden_states, f).reshape(TOK, H)
    xT = np.ascontiguousarray(X.T)

    pos = np.asarray(position_ids).astype(f).reshape(TOK)
    inv_freq = (1.0 / (THETA ** (np.arange(0, HD, 2, dtype=f) / HD))).astype(f)
    M = inv_freq[:, None] * pos[None, :]              # [64, TOK]
    cosT = np.repeat(np.cos(M), 2, axis=0).astype(f)  # [128, TOK]
    sinT = np.repeat(np.sin(M), 2, axis=0).astype(f)

    rotM = np.zeros((HD, HD), f)
    for i in range(HD // 2):
        rotM[2 * i + 1, 2 * i] = -1.0   # out[2i]   = -in[2i+1]
        rotM[2 * i, 2 * i + 1] = 1.0    # out[2i+1] =  in[2i]

    shared = {
        "xT": xT, "cosT": cosT, "sinT": sinT, "rotM": rotM,
        "ident": np.eye(128, dtype=f), "ones": np.ones((128, 128), f),
        "bo8": (np.asarray(bo, f) / NCORES).reshape(1, H),
    }
    Wq, Wk, Wv, Wo = (np.asarray(a, f) for a in (Wq, Wk, Wv, Wo))
    bq, bk, bv = (np.asarray(a, f) for a in (bq, bk, bv))
    in_maps = []
    for c in range(NCORES):
        m = dict(shared)
        m["wqT"] = np.ascontiguousarray(Wq[c * QO:(c + 1) * QO, :].T)
        m["wkT"] = np.ascontiguousarray(Wk[c * HD:(c + 1) * HD, :].T)
        m["wvT"] = np.ascontiguousarray(Wv[c * HD:(c + 1) * HD, :].T)
        m["woT"] = np.ascontiguousarray(Wo[:, c * QO:(c + 1) * QO].T)
        m["bqP"] = np.ascontiguousarray(bq[c * QO:(c + 1) * QO].reshape(QH, 128).T)
        m["bkP"] = bk[c * HD:(c + 1) * HD].reshape(128, 1).copy()
        m["bvP"] = bv[c * HD:(c + 1) * HD].reshape(128, 1).copy()
        in_maps.append(m)
    return in_maps


def kernel(hidden_states, position_ids, Wq, bq, Wk, bk, Wv, bv, Wo, bo):
    global LAST_EXEC_NS, LAST_RESULT
    from concourse.bass_utils import run_bass_kernel_spmd

    if "nc" not in _compiled:
        _compiled["nc"] = _build()
    nc = _compiled["nc"]

    in_maps = _host_inputs(hidden_states, position_ids,
                           Wq, bq, Wk, bk, Wv, bv, Wo, bo)
    trace = os.environ.get("KERNEL_TRACE", "0") == "1"
    res = run_bass_kernel_spmd(nc, in_maps, core_ids=list(range(NCORES)),
                               trace=trace)
    LAST_EXEC_NS = res.exec_time_ns
    LAST_RESULT = res
    full = np.concatenate([res.results[c]["out"] for c in range(NCORES)], axis=0)
    return full.reshape(B, S, H)


# revision 14
# speedup vs baseline: 1.8952x; 1.8952x over previous
"""Trainium2 Bass kernel for Llama-style GQA attention (B=2,S=2048,H=4096,NH=32,NKV=8,HD=128).

Sharding: tensor-parallel over heads — core c owns Q-heads 4c..4c+3 and GQA KV-head c
(Wq/Wk/Wv column-parallel, Wo row-parallel), ReduceScatter over token rows for the
output projection. kernel(**inputs) takes full inputs, returns the full output.
"""

import math
import os
from contextlib import ExitStack

import numpy as np

B, S, H = 2, 2048, 4096
NH, NKV, HD = 32, 8, 128
THETA = 1000000.0
NCORES = 8
QH = NH // NCORES            # 4 q-heads per core
TOK = B * S                  # 4096 tokens (flattened batch*seq)
QO = QH * HD                 # 512 q-out dims per core
TT = TOK // 128              # 32 token tiles of 128
TS = TOK // 512              # 8 token slices of 512
SB = S // 512                # 4 q-slices of 512 per batch
KTB = S // 128               # 16 k-tiles of 128 per batch
F32 = None                   # set after mybir import

LAST_EXEC_NS = None
LAST_RESULT = None

_compiled = {}


def _build():
    import concourse.bass as bass
    import concourse.mybir as mybir
    import concourse.tile as tile
    from concourse import bacc

    f32 = mybir.dt.float32
    f32r = mybir.dt.float32r            # fp32 w/ 11-bit mantissa: 1 PE cyc/row
    nc = bacc.Bacc("TRN2", target_bir_lowering=False, debug=False,
                   num_devices=NCORES)

    def inp(name, shape, dt=f32):
        return nc.dram_tensor(name, shape, dt, kind="ExternalInput").ap()

    xT = inp("xT", (H, TOK), f32r)      # hidden transposed
    wqT = inp("wqT", (H, QO), f32r)     # Wq shard, pre-transposed (k=h on rows)
    wkT = inp("wkT", (H, HD), f32r)
    wvT = inp("wvT", (H, HD), f32r)
    woT = inp("woT", (QO, H), f32r)     # Wo shard transposed (k=o on rows)
    bqP = inp("bqP", (128, QH))         # bq shard as [d, head]
    bkP = inp("bkP", (128, 1))
    bvP = inp("bvP", (128, 1))
    bo8 = inp("bo8", (1, H))            # bo / 8
    cosT = inp("cosT", (HD, TOK), f32r)
    sinT = inp("sinT", (HD, TOK))
    rotM = inp("rotM", (HD, HD), f32r)  # lhsT for rotate_half_interleaved
    ident = inp("ident", (128, 128), f32r)
    ones = inp("ones", (128, 128), f32r)
    maskI = inp("maskI", (128, 4, 512), f32r)  # causal diag-tile masks

    out = nc.dram_tensor("out", (TOK // NCORES, H), f32, kind="ExternalOutput").ap()
    partial = nc.dram_tensor("partial", (TOK, H), f32, kind="Internal").ap()
    rs_out = nc.dram_tensor("rs_out", (TOK // NCORES, H), f32,
                            kind="Internal").ap()

    inv_sqrt_hd = 1.0 / math.sqrt(HD)

    def mm(out, lhsT, rhs, **kw):
        nc.tensor.matmul(out, lhsT, rhs, **kw)

    with tile.TileContext(nc) as tc, ExitStack() as stk:
        # ---------------- constants + persistent activations ----------------
        cpool = stk.enter_context(tc.tile_pool(name="consts", bufs=1))
        apool = stk.enter_context(tc.tile_pool(name="acts", bufs=1))

        cos_sb = cpool.tile([128, TOK], f32r)
        nc.sync.dma_start(cos_sb[:], cosT[:])
        sin_sb = cpool.tile([128, TOK], f32)
        nc.sync.dma_start(sin_sb[:], sinT[:])
        rot_sb = cpool.tile([128, 128], f32r)
        nc.sync.dma_start(rot_sb[:], rotM[:])
        id_sb = cpool.tile([128, 128], f32r)
        nc.sync.dma_start(id_sb[:], ident[:])
        ones_sb = cpool.tile([128, 128], f32r)
        nc.sync.dma_start(ones_sb[:], ones[:])
        bq_sb = cpool.tile([128, QH], f32)
        nc.sync.dma_start(bq_sb[:], bqP[:])
        bk_sb = cpool.tile([128, 1], f32)
        nc.sync.dma_start(bk_sb[:], bkP[:])
        bv_sb = cpool.tile([128, 1], f32)
        nc.sync.dma_start(bv_sb[:], bvP[:])
        bo8_sb = cpool.tile([1, H], f32)
        nc.sync.dma_start(bo8_sb[:], bo8[:])
        # bo/8 broadcast to all partitions, used in the O-proj PSUM drain
        bo_bc = cpool.tile([128, H], f32)
        nc.gpsimd.partition_broadcast(bo_bc[:], bo8_sb[:])

        # causal masks for the 4 diagonal 128x512 tiles: keep iff n - p - 128a >= 0
        mask_sb = cpool.tile([128, 4, 512], f32r)
        nc.sync.dma_start(mask_sb[:], maskI[:])

        KT = apool.tile([128, TOK], f32r)        # K^T (rope'd), grows causally
        Vsb = apool.tile([128, TT, 128], f32r)   # V in [t mod 128, t tile, d]

        # single fused causal loop over 512-token slices; one shared PSUM tag
        sp = stk.enter_context(tc.tile_pool(name="streams", bufs=3))
        tp = stk.enter_context(tc.tile_pool(name="tmps", bufs=2))
        qtp = stk.enter_context(tc.tile_pool(name="qts", bufs=2))
        vtp = stk.enter_context(tc.tile_pool(name="vts", bufs=2))
        atp = stk.enter_context(tc.tile_pool(name="attw", bufs=4))
        smp = stk.enter_context(tc.tile_pool(name="smalls", bufs=2))
        anp = stk.enter_context(tc.tile_pool(name="atn", bufs=2))
        stp = stk.enter_context(tc.tile_pool(name="ostage", bufs=2))
        wop = stk.enter_context(tc.tile_pool(name="wo", bufs=2))
        pp = stk.enter_context(tc.tile_pool(name="ps", bufs=8, space="PSUM"))

        def ps_tile(shape=(128, 512)):
            return pp.tile(list(shape), f32, name="ps", tag="ps")

        for ti in range(TS):
            b, j = ti // SB, ti % SB
            t0 = ti * 512
            # ---- QKV projection for this token slice (accumulate over h) ----
            psq = [ps_tile() for _ in range(QH)]
            psk = ps_tile()
            psv = ps_tile()
            for hi in range(H // 128):
                h0 = hi * 128
                xt = sp.tile([128, 512], f32r, name="xt")
                nc.sync.dma_start(xt[:], xT[h0:h0 + 128, t0:t0 + 512])
                wq_t = sp.tile([128, QO], f32r, name="wq_t")
                nc.sync.dma_start(wq_t[:], wqT[h0:h0 + 128, :])
                wk_t = sp.tile([128, HD], f32r, name="wk_t")
                nc.sync.dma_start(wk_t[:], wkT[h0:h0 + 128, :])
                wv_t = sp.tile([128, HD], f32r, name="wv_t")
                nc.sync.dma_start(wv_t[:], wvT[h0:h0 + 128, :])
                st = (hi == 0)
                en = (hi == H // 128 - 1)
                for q in range(QH):
                    mm(psq[q][:], wq_t[:, q * 128:(q + 1) * 128],
                       xt[:], start=st, stop=en)
                mm(psk[:], wk_t[:], xt[:], start=st, stop=en)
                mm(psv[:], wv_t[:], xt[:], start=st, stop=en)

            # bias add (per-partition) while draining PSUM
            QTs = qtp.tile([128, QH, 512], f32r, name="QTs")
            VTs = vtp.tile([128, 512], f32r, name="VTs")
            for q in range(QH):
                nc.scalar.add(QTs[:, q, :], psq[q][:], bq_sb[:, q:q + 1])
            nc.scalar.add(KT[:, t0:t0 + 512], psk[:], bk_sb[:, 0:1])
            nc.scalar.add(VTs[:], psv[:], bv_sb[:, 0:1])

            # rope in place on QT / KT slices
            def rope(ap_slice):
                rps = ps_tile()
                mm(rps[:], rot_sb[:], ap_slice, start=True, stop=True)
                t1 = tp.tile([128, 512], f32, name="t1")
                nc.vector.tensor_mul(t1[:], ap_slice, cos_sb[:, t0:t0 + 512])
                t2 = tp.tile([128, 512], f32, name="t2")
                nc.vector.tensor_mul(t2[:], rps[:], sin_sb[:, t0:t0 + 512])
                nc.vector.tensor_add(ap_slice, t1[:], t2[:])

            for q in range(QH):
                rope(QTs[:, q, :])
            rope(KT[:, t0:t0 + 512])

            # V^T -> V (PE transpose of 128x128 blocks)
            for s4 in range(4):
                g = ti * 4 + s4
                vps = pp.tile([128, 128], f32r, name="vps", tag="ps")
                nc.tensor.transpose(vps[:], VTs[:, s4 * 128:(s4 + 1) * 128],
                                    id_sb[:])
                nc.scalar.copy(Vsb[:, g, :], vps[:])

            # ---- causal attention for this q slice ----
            nk = 4 * j + 4                # k tiles of 128 within batch b
            ATn = anp.tile([128, QH, 512], f32r, name="ATn")
            for h in range(QH):
                dn_ps = ps_tile((1, 512))
                at_ps = ps_tile()
                for ki in range(nk):
                    kg = b * KTB + ki
                    sc_ps = ps_tile()
                    mm(sc_ps[:], KT[:, kg * 128:(kg + 1) * 128],
                       QTs[:, h, :], start=True, stop=True)
                    a_sb = atp.tile([128, 512], f32r, name="a_sb")
                    nc.scalar.activation(a_sb[:], sc_ps[:],
                                         mybir.ActivationFunctionType.Exp,
                                         scale=inv_sqrt_hd)
                    if ki >= 4 * j:
                        nc.vector.tensor_mul(a_sb[:], a_sb[:],
                                             mask_sb[:, ki - 4 * j, :])
                    mm(dn_ps[:], ones_sb[:, 0:1], a_sb[:],
                       start=(ki == 0), stop=(ki == nk - 1))
                    mm(at_ps[:], Vsb[:, kg, :], a_sb[:],
                       start=(ki == 0), stop=(ki == nk - 1))
                dr = smp.tile([1, 512], f32, name="dr")
                nc.vector.reciprocal(dr[:], dn_ps[:])
                rb = smp.tile([128, 512], f32, name="rb")
                nc.gpsimd.partition_broadcast(rb[:], dr[:])
                nc.vector.tensor_mul(ATn[:, h, :], at_ps[:], rb[:])

            # ---- O-projection for this 512-token slice (row-parallel Wo) ----
            for f in range(H // 512):
                f0 = f * 512
                wo_t = wop.tile([128, QH, 512], f32r, name="wo_t")
                for o in range(QH):
                    nc.sync.dma_start(wo_t[:, o, :],
                                      woT[o * 128:(o + 1) * 128, f0:f0 + 512])
                for t4 in range(4):
                    op_ps = ps_tile()
                    for o in range(QH):
                        mm(op_ps[:], ATn[:, o, t4 * 128:(t4 + 1) * 128],
                           wo_t[:, o, :], start=(o == 0), stop=(o == QH - 1))
                    st_t = stp.tile([128, 512], f32, name="st_t")
                    nc.vector.tensor_add(st_t[:], op_ps[:], bo_bc[:, f0:f0 + 512])
                    nc.sync.dma_start(
                        partial[t0 + t4 * 128:t0 + (t4 + 1) * 128, f0:f0 + 512],
                        st_t[:])

        # ---------------- phase 3: reduce-scatter + output -------------------
        nc.gpsimd.collective_compute(
            "ReduceScatter", mybir.AluOpType.add,
            replica_groups=[list(range(NCORES))],
            ins=[partial.opt()], outs=[rs_out.opt()],
        )
        nc.sync.dma_start(out[:], rs_out[:])

    nc.compile()
    return nc


def _fp32r(x):
    """Round fp32 to fp32r (11-bit mantissa, RTNE, low 12 bits zero)."""
    u = np.ascontiguousarray(x, np.float32).view(np.uint32)
    lsb = (u >> 12) & 1
    out = ((u + 0x7FF + lsb) & np.uint32(0xFFFFF000)).view(np.float32)
    return out


def _host_inputs(hidden_states, position_ids, Wq, bq, Wk, bk, Wv, bv, Wo, bo):
    f = np.float32
    X = np.asarray(hidden_states, f).reshape(TOK, H)
    xT = _fp32r(np.ascontiguousarray(X.T))

    pos = np.asarray(position_ids).astype(f).reshape(TOK)
    inv_freq = (1.0 / (THETA ** (np.arange(0, HD, 2, dtype=f) / HD))).astype(f)
    M = inv_freq[:, None] * pos[None, :]              # [64, TOK]
    cosT = np.repeat(np.cos(M), 2, axis=0).astype(f)  # [128, TOK]
    sinT = np.repeat(np.sin(M), 2, axis=0).astype(f)

    rotM = np.zeros((HD, HD), f)
    for i in range(HD // 2):
        rotM[2 * i + 1, 2 * i] = -1.0   # out[2i]   = -in[2i+1]
        rotM[2 * i, 2 * i + 1] = 1.0    # out[2i+1] =  in[2i]

    shared = {
        "xT": xT, "cosT": _fp32r(cosT), "sinT": sinT, "rotM": rotM,
        "ident": np.eye(128, dtype=f), "ones": np.ones((128, 128), f),
        "bo8": (np.asarray(bo, f) / NCORES).reshape(1, H),
        "maskI": (np.arange(512)[None, None, :]
                  - np.arange(128)[:, None, None]
                  - 128 * np.arange(4)[None, :, None] >= 0).astype(f),
    }
    Wq, Wk, Wv, Wo = (np.asarray(a, f) for a in (Wq, Wk, Wv, Wo))
    bq, bk, bv = (np.asarray(a, f) for a in (bq, bk, bv))
    in_maps = []
    for c in range(NCORES):
        m = dict(shared)
        m["wqT"] = _fp32r(Wq[c * QO:(c + 1) * QO, :].T)
        m["wkT"] = _fp32r(Wk[c * HD:(c + 1) * HD, :].T)
        m["wvT"] = _fp32r(Wv[c * HD:(c + 1) * HD, :].T)
        m["woT"] = _fp32r(Wo[:, c * QO:(c + 1) * QO].T)
        m["bqP"] = np.ascontiguousarray(bq[c * QO:(c + 1) * QO].reshape(QH, 128).T)
        m["bkP"] = bk[c * HD:(c + 1) * HD].reshape(128, 1).copy()
        m["bvP"] = bv[c * HD:(c + 1) * HD].reshape(128, 1).copy()
        in_maps.append(m)
    return in_maps


def kernel(hidden_states, position_ids, Wq, bq, Wk, bk, Wv, bv, Wo, bo):
    global LAST_EXEC_NS, LAST_RESULT
    from concourse.bass_utils import run_bass_kernel_spmd

    if "nc" not in _compiled:
        _compiled["nc"] = _build()
    nc = _compiled["nc"]

    in_maps = _host_inputs(hidden_states, position_ids,
                           Wq, bq, Wk, bk, Wv, bv, Wo, bo)
    trace = os.environ.get("KERNEL_TRACE", "0") == "1"
    res = run_bass_kernel_spmd(nc, in_maps, core_ids=list(range(NCORES)),
                               trace=trace)
    LAST_EXEC_NS = res.exec_time_ns
    LAST_RESULT = res
    full = np.concatenate([res.results[c]["out"] for c in range(NCORES)], axis=0)
    return full.reshape(B, S, H)
